# revision 17
# baseline (speedup 1.0000x reference)
"""Trainium2 Bass kernel for nn_Pooling_Layer (GNN message-passing pooling).

Math (per batch element b):
    x = in_pc_pad[b] @ weight_res.T               # (N+1, 64) -> (N+1, 128) projection
    w = |p_neighbors| * mask; w /= w.sum(-1)+1e-8 # (P, 32) pooling weights
    out[b, p] = sum_m w[p, m] * x[id[p, m]]       # gather + weighted pool

We reorder: pool first in C_IN=64 space (gather is half the bytes), then
project pooled (P, 64) @ weight_res.T.  Normalization (divide by the weight
sum) is folded into the PSUM->SBUF copy after the projection.

Sharding: points are sharded across the 8 cores (1250 points each); every
core handles ALL batches for its points.  The gather table holds row PAIRS,
batch-interleaved, in bf16: xi[k] = [row 2k: b0..b7 x 64ch | row 2k+1:
b0..b7 x 64ch] (2KB rows).  Pairs keep the SWDGE gather indices int16-safe
(idx = id >> 1 <= 20000); one descriptor serves all 8 batches at a
DMA-efficient 2KB.  bf16 halves HBM gather traffic vs f32; the tolerance
(2e-2) dwarfs bf16 rounding (~0.5%).

Pooling runs on the TensorEngine: per 128-point tile, 64 accumulating bf16
matmuls (32 windows x even/odd half) into one (128 pts, 8b*64ch) PSUM bank.
lhsT is a block-diagonal weight matrix with a FIXED sparsity structure:
window w (slots = partitions: slot 32q+m = neighbor m of point 4w+q) puts
weight at [32q+m, 4w+q].  Even-half weights are |pn|*mask*(1-parity), odd
|pn|*mask*parity, so the wrong half of each gathered pair contributes 0.
The nonzero positions are identical for every tile, so the bd buffers are
zeroed once and only the values are rewritten per tile (tiny strided
copies).

Then per tile: 4 PE transposes (128pts, 2 batches*64ch) -> (128ch, 128pts),
8 projection matmuls lhsT=pooled^T (64,128) rhs=weight_res^T (64,128), and
the per-point 1/denom scale on the PSUM->SBUF copy.  Output is bf16,
upcast and re-assembled on the host.
"""

import numpy as np
import ml_dtypes

import concourse.bass as bass
import concourse.mybir as mybir
import concourse.tile as tile
from concourse import bacc, library_config
from concourse.bass_utils import run_bass_kernel_spmd

F32 = mybir.dt.float32
BF16 = mybir.dt.bfloat16
I16 = mybir.dt.int16

MAXN = 32
CIN = 64
COUT = 128
B = 8
IN_ROWS = 40001          # in_pc_pad rows (incl. pad row)
NPAIRS = 20001           # row pairs (rows padded to 40002)
EW = B * CIN             # interleaved single-row width (elements) = 512
PEW = 2 * EW             # pair-row width = 1024 elements (2KB bf16)
PTS = 10000
NWIN = 32                # windows (4-point groups) per 128-point tile
CHW = 8                  # windows per gather call (1024 idx)
NCALL = (NWIN + CHW - 1) // CHW   # gather call slots per tile


class Params:
    def __init__(self, pts=PTS, n_cores=8):
        self.pts = pts
        self.n_cores = n_cores
        self.cpts = pts // n_cores            # points per core (1250)
        self.ntl = (self.cpts + 127) // 128   # 128-point tiles per core (10)
        self.cpts_pad = self.ntl * 128        # 1280


def build_nc(p: Params):
    nc = bacc.Bacc(
        "TRN2",
        target_bir_lowering=False,
        debug=False,
        num_devices=p.n_cores,
        num_swdge_queues=4,
    )
    NTL = p.ntl
    xi = nc.dram_tensor("xi", [NPAIRS, PEW], BF16, kind="ExternalInput")
    idxw = nc.dram_tensor("idxw", [128, NTL * NWIN * 8], I16, kind="ExternalInput")
    pnT = nc.dram_tensor("pnT", [128, p.cpts_pad], F32, kind="ExternalInput")
    maskT = nc.dram_tensor("maskT", [128, p.cpts_pad], F32, kind="ExternalInput")
    parT = nc.dram_tensor("parT", [128, p.cpts_pad], F32, kind="ExternalInput")
    pnN = nc.dram_tensor("pnN", [p.cpts_pad, MAXN], F32, kind="ExternalInput")
    maskN = nc.dram_tensor("maskN", [p.cpts_pad, MAXN], F32, kind="ExternalInput")
    wres = nc.dram_tensor("wres", [COUT, CIN], F32, kind="ExternalInput")
    ident = nc.dram_tensor("ident", [128, 128], F32, kind="ExternalInput")
    out = nc.dram_tensor("out", [B * p.cpts_pad, COUT], BF16, kind="ExternalOutput")

    with tile.TileContext(nc) as tc:
        with (
            tc.tile_pool(name="const", bufs=1) as constp,
            tc.tile_pool(name="prep", bufs=1) as prep,
            tc.tile_pool(name="gather", bufs=7) as gp,
            tc.tile_pool(name="work", bufs=2) as wk,
            tc.tile_pool(name="psP", bufs=2, space="PSUM") as psP,
            tc.tile_pool(name="psT", bufs=2, space="PSUM") as psT,
            tc.tile_pool(name="psO", bufs=2, space="PSUM") as psO,
        ):
            nc.gpsimd.load_library(library_config.mlp)

            # ---- constants ----
            identity = constp.tile([128, 128], F32)
            nc.sync.dma_start(out=identity[:], in_=ident[:])
            wres_sb = constp.tile([COUT, CIN], F32)
            nc.sync.dma_start(out=wres_sb[:], in_=wres[:])
            psw = psT.tile([CIN, COUT], F32, tag="psTt")
            nc.tensor.transpose(out=psw[:], in_=wres_sb[:], identity=identity[:])
            # [i, o] = wres[o, i], replicated into both 64-partition halves so
            # the projection matmul's rhs base partition matches lhsT's
            wresTb = constp.tile([128, COUT], BF16)
            nc.vector.tensor_copy(out=wresTb[0:CIN, :], in_=psw[:])
            nc.vector.tensor_copy(out=wresTb[CIN : 2 * CIN, :], in_=psw[:])

            # idx loaded per tile so the first gather starts immediately
            idx_sb = constp.tile([128, NTL * NWIN * 8], I16)
            for t in range(NTL):
                c0 = t * NWIN * 8
                nc.sync.dma_start(
                    out=idx_sb[:, c0 : c0 + NWIN * 8],
                    in_=idxw[:, c0 : c0 + NWIN * 8],
                )

            # ---- per-point reciprocal denominators: recip[p, t] ----
            prodN = prep.tile([128, NTL * MAXN], F32)
            nc.sync.dma_start(
                out=prodN[:].rearrange("p (t m) -> p t m", m=MAXN),
                in_=pnN[:].rearrange("(t p) m -> p t m", p=128),
            )
            maskN_sb = prep.tile([128, NTL * MAXN], F32)
            nc.sync.dma_start(
                out=maskN_sb[:].rearrange("p (t m) -> p t m", m=MAXN),
                in_=maskN[:].rearrange("(t p) m -> p t m", p=128),
            )
            nc.vector.tensor_tensor(
                out=prodN[:], in0=prodN[:], in1=maskN_sb[:], op=mybir.AluOpType.mult
            )
            denom = constp.tile([128, NTL], F32)
            nc.vector.tensor_reduce(
                out=denom[:],
                in_=prodN[:].rearrange("p (t m) -> p t m", m=MAXN),
                op=mybir.AluOpType.add,
                axis=mybir.AxisListType.X,
                apply_absolute_value=True,
            )
            nc.vector.tensor_scalar_add(denom[:], denom[:], 1e-8)
            recip = constp.tile([128, NTL], F32)
            nc.vector.reciprocal(out=recip[:], in_=denom[:])

            # ---- pooling weights in (32q+m, pt) layout ----
            # wsel0 = |pn|*mask*(1-par)   (even half)
            # wsel1 = |pn|*mask*par       (odd half)
            pnT_sb = prep.tile([128, p.cpts_pad], F32)
            maskT_sb = prep.tile([128, p.cpts_pad], F32)
            parT_sb = prep.tile([128, p.cpts_pad], F32)
            nc.sync.dma_start(out=pnT_sb[:], in_=pnT[:])
            nc.sync.dma_start(out=maskT_sb[:], in_=maskT[:])
            nc.sync.dma_start(out=parT_sb[:], in_=parT[:])
            wsel0 = prep.tile([128, p.cpts_pad], F32)
            wsel1 = prep.tile([128, p.cpts_pad], F32)
            nc.scalar.activation(
                out=wsel0[:], in_=pnT_sb[:], func=mybir.ActivationFunctionType.Abs
            )
            nc.vector.tensor_tensor(
                out=wsel0[:], in0=wsel0[:], in1=maskT_sb[:], op=mybir.AluOpType.mult
            )
            nc.vector.tensor_tensor(
                out=wsel1[:], in0=wsel0[:], in1=parT_sb[:], op=mybir.AluOpType.mult
            )
            nc.vector.tensor_tensor(
                out=wsel0[:], in0=wsel0[:], in1=wsel1[:], op=mybir.AluOpType.subtract
            )

            # ---- block-diag weight buffers: fixed sparsity, zeroed once ----
            BDW = NWIN * 132  # 4224: bd[s, 132w + q] == lhsT col 4w+q of window w
            bd_bufs = []      # [t%2][half] ping-pong pairs
            for i in range(2):
                pair = []
                for half in range(2):
                    bdt = constp.tile([128, BDW], BF16, tag=f"bd{i}h{half}")
                    nc.vector.memset(bdt[:], 0.0)
                    pair.append(bdt)
                bd_bufs.append(pair)

            # ---- main loop over 128-point tiles ----
            for t in range(NTL):
                # windows with at least one real (non-pad) point
                real_pts = min(128, p.cpts - t * 128)
                nwin_t = (real_pts + 3) // 4
                # scatter this tile's weights onto the fixed block-diag slots
                bde, bdo = bd_bufs[t % 2]
                for bd, src in ((bde, wsel0), (bdo, wsel1)):
                    bdv = bd[:].rearrange("p (w c) -> p w c", c=132)
                    sv = src[:, t * 128 : (t + 1) * 128].rearrange(
                        "p (w four) -> p w four", four=4
                    )
                    for q in range(4):
                        nc.vector.tensor_copy(
                            out=bdv[32 * q : 32 * q + 32, :, q],
                            in_=sv[32 * q : 32 * q + 32, :, q],
                        )

                # gather + pool in chunks of 8 windows (1024 idx per call)
                ps = psP.tile([128, EW], F32, tag="ps")
                for c in range((nwin_t + CHW - 1) // CHW):
                    nw_c = min(CHW, nwin_t - c * CHW)
                    g = gp.tile([128, CHW * PEW], BF16, tag="g")
                    call = t * NCALL + c
                    col0 = t * NWIN * 8 + c * CHW * 8
                    nc.gpsimd.dma_gather(
                        g[:, : nw_c * PEW].rearrange("p (v e) -> p v e", e=PEW),
                        xi[:],
                        idx_sb[:, col0 : col0 + nw_c * 8],
                        nw_c * 128,
                        nw_c * 128,
                        PEW,
                        queue_num=call % 4,
                    )
                    for v in range(nw_c):
                        w = c * CHW + v
                        for half, bd in ((0, bde), (1, bdo)):
                            nc.tensor.matmul(
                                out=ps[:],
                                lhsT=bd[:, w * 128 : w * 128 + 128],
                                rhs=g[
                                    :,
                                    v * PEW + half * EW : v * PEW + (half + 1) * EW,
                                ],
                                start=(w == 0 and half == 0),
                                stop=(w == nwin_t - 1 and half == 1),
                            )
                pooled = wk.tile([128, EW], F32, tag="pooled")
                nc.scalar.copy(out=pooled[:], in_=ps[:])

                # transpose 2-batch blocks, project, scale by 1/denom, store
                for k in range(4):
                    pst = psT.tile([128, 128], F32, tag="psTt")
                    nc.tensor.transpose(
                        out=pst[:],
                        in_=pooled[:, k * 128 : (k + 1) * 128],
                        identity=identity[:],
                    )
                    poolTb = wk.tile([128, 128], BF16, tag="poolTb")
                    nc.vector.tensor_copy(out=poolTb[:], in_=pst[:])
                    for h in range(2):
                        b = 2 * k + h
                        pso = psO.tile([128, COUT], F32, tag="psO")
                        nc.tensor.matmul(
                            out=pso[:],
                            lhsT=poolTb[64 * h : 64 * h + 64, :],
                            rhs=wresTb[64 * h : 64 * h + 64, :],
                            start=True,
                            stop=True,
                        )
                        outP = wk.tile([128, COUT], BF16, tag="outP")
                        nc.vector.tensor_scalar_mul(
                            outP[:], pso[:], recip[:, t : t + 1]
                        )
                        r0 = b * p.cpts_pad + t * 128
                        nc.sync.dma_start(out=out[r0 : r0 + 128, :], in_=outP[:])
    nc.compile()
    return nc


def host_prep(p: Params, in_pc_pad, ids, mask, pn, wres):
    """Per-core input maps.  Host work is layout marshalling only."""
    ids = np.asarray(ids).astype(np.int64)
    pn = np.asarray(pn, dtype=np.float32)
    mask = np.asarray(mask, dtype=np.float32)
    wres = np.asarray(wres, dtype=np.float32)
    x = np.asarray(in_pc_pad, dtype=np.float32)          # (B, 40001, 64)

    # pair table: xi[k] = [row 2k all batches | row 2k+1 all batches], bf16
    xp = np.concatenate([x, np.zeros((B, 1, CIN), np.float32)], axis=1)
    xi = np.ascontiguousarray(
        xp.transpose(1, 0, 2).reshape(2 * NPAIRS, EW).reshape(NPAIRS, PEW)
    ).astype(ml_dtypes.bfloat16)
    ident = np.eye(128, dtype=np.float32)

    in_maps = []
    for c in range(p.n_cores):
        lo = c * p.cpts

        def pad_pts(a, dtype):
            o = np.zeros((p.cpts_pad, MAXN), dtype=dtype)
            o[: p.cpts] = a[lo : lo + p.cpts]
            return o

        ids_c = pad_pts(ids, np.int64)
        ids_c[p.cpts :] = 2 * (NPAIRS - 1)               # pad points: valid pair
        pn_c = pad_pts(pn, np.float32)
        mask_c = pad_pts(mask, np.float32)
        par_c = (ids_c & 1).astype(np.float32)
        idx16 = (ids_c >> 1).astype(np.int16)

        # gather stream: tile t, window w, slot s=32q+m -> ids_c[t*128+4w+q, m]>>1
        flat = (
            idx16.reshape(p.ntl, NWIN, 4, MAXN)
            .transpose(0, 1, 2, 3)                       # (t, w, q, m)
            .reshape(p.ntl * NWIN * 128)
        )
        # wrapped-16 layout per call: idx i at [i % 16, i // 16]
        idx_w = np.zeros((128, p.ntl * NWIN * 8), np.int16)
        for t in range(p.ntl):
            for c in range(NCALL):
                w0 = c * CHW
                nwn = min(CHW, NWIN - w0)
                blk_flat = flat[t * 4096 + w0 * 128 : t * 4096 + (w0 + nwn) * 128]
                blk = blk_flat.reshape(nwn * 8, 16).T
                col0 = t * NWIN * 8 + c * CHW * 8
                idx_w[:, col0 : col0 + nwn * 8] = np.tile(blk, (8, 1))

        pnT = np.ascontiguousarray(np.tile(pn_c.T, (4, 1)))      # (128, cpts_pad)
        maskT = np.ascontiguousarray(np.tile(mask_c.T, (4, 1)))
        parT = np.ascontiguousarray(np.tile(par_c.T, (4, 1)))
        in_maps.append(
            {
                "xi": xi,
                "idxw": idx_w,
                "pnT": pnT,
                "maskT": maskT,
                "parT": parT,
                "pnN": pn_c,
                "maskN": mask_c,
                "wres": wres,
                "ident": ident,
            }
        )
    return in_maps


def assemble(p: Params, results):
    out = np.empty((B, p.pts, COUT), np.float32)
    for c in range(p.n_cores):
        got = np.asarray(results[c]["out"], dtype=np.float32).reshape(
            B, p.cpts_pad, COUT
        )
        out[:, c * p.cpts : (c + 1) * p.cpts, :] = got[:, : p.cpts, :]
    return out


_NC_CACHE = {}


def get_nc(p: Params):
    key = (p.pts, p.n_cores)
    if key not in _NC_CACHE:
        _NC_CACHE[key] = build_nc(p)
    return _NC_CACHE[key]


def kernel(in_pc_pad, neighbor_id_lstlst, neighbor_mask_lst, p_neighbors, weight_res):
    in_pc_pad = np.asarray(in_pc_pad)
    p = Params(pts=PTS, n_cores=in_pc_pad.shape[0])
    in_maps = host_prep(
        p, in_pc_pad, neighbor_id_lstlst, neighbor_mask_lst, p_neighbors, weight_res
    )
    nc = get_nc(p)
    res = run_bass_kernel_spmd(nc, in_maps, core_ids=list(range(p.n_cores)))
    return assemble(p, res.results)
